# revision 37
# baseline (speedup 1.0000x reference)
"""Ensemble MLP surrogate (16 models, 32->64->64->64->8, relu) on 8 TRN2 cores.

Strategy (data-parallel over batch, weights replicated; streaming-bound PE):
  host packs x transposed + 4x row-replicated [128, B/8] per core in fp16;
  feature-on-partition layout, batch streams as the matmul moving operand.
  L1 runs one K=32,M=128 matmul per pair (both models' W1 side by side) on
  row-group j%4 so pairs can overlap; L2/L3 are single K=128,M=128 pair
  block-diagonal matmuls (half the instruction count of 2x2 quads at the
  same stream cost); L4 packs 4 pairs per PSUM bank via column offsets in
  a dense end-of-tile phase.  Ensemble mean / sum-of-squares run on the PE
  via a shared 1/16 selector; stats for (dtile,half) land at PSUM column
  position 32q so 2 dtiles of stats drain with one set of fd=512 ops.
  Bias+ReLU epilogues (PSUM->SBUF) are load-balanced across Vector and
  Scalar; squares and mean^2 run on the otherwise-idle GpSimd engine.
"""

import numpy as np

N_MODELS = 16
IN_DIM = 32
HID = 64
OUT_DIM = 8
BATCH = 131072
N_CORES = 8
B_CORE = BATCH // N_CORES  # 16384
TILE = 512  # matmul moving-operand columns (fp32 PSUM bank limit on out)
DTILE = 2 * TILE  # batch elements per pipeline step
NPAIR = N_MODELS // 2

# wpackr free-dim layout (fp16 matmul operands, 128 partitions)
OFF_W1 = 0  # [128, 8, 128]: rows 32k hold [W1_2j | W1_2j+1] (4 replicas)
OFF_W2 = OFF_W1 + NPAIR * 128  # [128, 8, 128] pair blockdiag
OFF_W3 = OFF_W2 + NPAIR * 128  # [128, 8, 128] pair blockdiag
OFF_W4 = OFF_W3 + NPAIR * 128  # [128, 8, 32] pair blockdiag (cols 16-31 zero)
OFF_SEL = OFF_W4 + NPAIR * 32  # [128, 8] 1/16 selector (mean and sumsq)
WR = OFF_SEL + 8
# wpackb free-dim layout (fp32 biases)
OFF_B1 = 0  # [128, 8]
OFF_B2 = OFF_B1 + NPAIR  # [128, 8]
OFF_B3 = OFF_B2 + NPAIR  # [128, 8]
OFF_B4 = OFF_B3 + NPAIR  # [128, 2] (per L4 bank-group)
WB = OFF_B4 + 2


# measured op costs (ns) for greedy DVE/ACT load balancing (includes the
# ~100ns/op issue+semaphore overhead observed in traces)
def _act_cost(fd, psum_src=True):
    return ((311 if psum_src else 363) + fd) / 1.2


def _dve_cost(fd, psum_src=True):
    return ((198 if psum_src else 136) + fd) / 0.96


def pack_inputs(x, W1, b1, W2, b2, W3, b3, W4, b4, b_core=B_CORE, n_cores=N_CORES):
    """Host-side packing. Returns (xt_per_core list, wpackr fp16, wpackb f32)."""
    f32 = np.float32
    x = np.ascontiguousarray(x, dtype=f32)
    wpack = np.zeros((128, WR), f32)
    wpackb = np.zeros((128, WB), f32)

    w1v = wpack[:, OFF_W1 : OFF_W1 + NPAIR * 128].reshape(128, NPAIR, 128)
    w2v = wpack[:, OFF_W2 : OFF_W2 + NPAIR * 128].reshape(128, NPAIR, 128)
    w3v = wpack[:, OFF_W3 : OFF_W3 + NPAIR * 128].reshape(128, NPAIR, 128)
    w4v = wpack[:, OFF_W4 : OFF_W4 + NPAIR * 32].reshape(128, NPAIR, 32)
    for j in range(NPAIR):
        a, b = 2 * j, 2 * j + 1
        for k in range(4):  # x replicated on all 4 row groups
            w1v[32 * k : 32 * k + 32, j, 0:HID] = W1[a]
            w1v[32 * k : 32 * k + 32, j, HID:128] = W1[b]
        w2v[0:HID, j, 0:HID] = W2[a]
        w2v[HID:128, j, HID:128] = W2[b]
        w3v[0:HID, j, 0:HID] = W3[a]
        w3v[HID:128, j, HID:128] = W3[b]
        w4v[0:HID, j, 0:OUT_DIM] = W4[a]
        w4v[HID:128, j, OUT_DIM : 2 * OUT_DIM] = W4[b]

    selm = wpack[:, OFF_SEL : OFF_SEL + 8]
    b4v = wpackb[:, OFF_B4 : OFF_B4 + 2]
    for q in range(4):  # pair-within-group
        for c in range(2):  # model-within-pair
            for o in range(OUT_DIM):
                p = 32 * q + 8 * c + o
                selm[p, o] = 1.0 / 16.0  # exact in fp16; 16/15 applied at sqrt
                b4v[p, 0] = b4[2 * q + c, o]  # group A: pairs 0-3
                b4v[p, 1] = b4[2 * (q + 4) + c, o]  # group B: pairs 4-7
    for j in range(NPAIR):
        a, b = 2 * j, 2 * j + 1
        wpackb[0:HID, OFF_B1 + j] = b1[a]
        wpackb[HID:128, OFF_B1 + j] = b1[b]
        wpackb[0:HID, OFF_B2 + j] = b2[a]
        wpackb[HID:128, OFF_B2 + j] = b2[b]
        wpackb[0:HID, OFF_B3 + j] = b3[a]
        wpackb[HID:128, OFF_B3 + j] = b3[b]

    wpack16 = wpack.astype(np.float16)
    x16 = x.astype(np.float16)
    xt_per_core = []
    for c in range(n_cores):
        shard = x16[c * b_core : (c + 1) * b_core]  # [b_core, 32]
        xt = np.ascontiguousarray(np.tile(shard.T, (4, 1)))  # [128, b_core]
        xt_per_core.append(xt)
    return xt_per_core, wpack16, wpackb


def _emit(tc, ctx, xt, wr, wb, meant, stdt, b_core):
    import concourse.bass as bass  # noqa: F401
    from concourse import mybir

    nc = tc.nc
    f32 = mybir.dt.float32
    f16 = mybir.dt.float16
    AF = mybir.ActivationFunctionType
    ALU = mybir.AluOpType

    n_dt = b_core // DTILE

    consts = ctx.enter_context(tc.tile_pool(name="consts", bufs=1))
    xp = ctx.enter_context(tc.tile_pool(name="xp", bufs=4))
    h1p = ctx.enter_context(tc.tile_pool(name="h1p", bufs=4))
    h2p = ctx.enter_context(tc.tile_pool(name="h2p", bufs=4))
    h3p = ctx.enter_context(tc.tile_pool(name="h3p", bufs=10))
    prp = ctx.enter_context(tc.tile_pool(name="prp", bufs=10))
    sqp = ctx.enter_context(tc.tile_pool(name="sqp", bufs=10))
    msbp = ctx.enter_context(tc.tile_pool(name="msbp", bufs=2))
    m2p = ctx.enter_context(tc.tile_pool(name="m2p", bufs=2))
    nvp = ctx.enter_context(tc.tile_pool(name="nvp", bufs=2))
    ssbp = ctx.enter_context(tc.tile_pool(name="ssbp", bufs=2))
    # PSUM budget (8 banks): php 4x[128,512]=4 + p4p 2x[128,512]=2 + stats 2
    php = ctx.enter_context(tc.tile_pool(name="php", bufs=4, space="PSUM"))
    p4p = ctx.enter_context(tc.tile_pool(name="p4p", bufs=2, space="PSUM"))
    mstp = ctx.enter_context(tc.tile_pool(name="mstp", bufs=1, space="PSUM"))
    sstp = ctx.enter_context(tc.tile_pool(name="sstp", bufs=1, space="PSUM"))

    cw = consts.tile([128, WR], f16)
    nc.gpsimd.dma_start(out=cw, in_=wr)
    cwb = consts.tile([128, WB], f32)
    nc.gpsimd.dma_start(out=cwb, in_=wb)
    w1v = cw[:, OFF_W1 : OFF_W1 + NPAIR * 128].rearrange("p (j f) -> p j f", f=128)
    w2v = cw[:, OFF_W2 : OFF_W2 + NPAIR * 128].rearrange("p (j f) -> p j f", f=128)
    w3v = cw[:, OFF_W3 : OFF_W3 + NPAIR * 128].rearrange("p (j f) -> p j f", f=128)
    w4v = cw[:, OFF_W4 : OFF_W4 + NPAIR * 32].rearrange("p (j f) -> p j f", f=32)
    selm = cw[:, OFF_SEL : OFF_SEL + 8]

    # greedy engine balancer for PSUM->SBUF epilogues
    eng_ns = {"act": 0.0, "dve": 0.0}

    def epilogue(out, in_, bias, relu, psum_src=True):
        fd = out.free_size()
        if eng_ns["act"] + _act_cost(fd, psum_src) <= eng_ns["dve"] + _dve_cost(
            fd, psum_src
        ):
            eng_ns["act"] += _act_cost(fd, psum_src)
            nc.scalar.activation(
                out, in_, AF.Relu if relu else AF.Identity, bias=bias, scale=1.0
            )
        else:
            eng_ns["dve"] += _dve_cost(fd, psum_src)
            if relu:
                nc.vector.tensor_scalar(
                    out, in_, bias, 0.0, op0=ALU.add, op1=ALU.max
                )
            else:
                nc.vector.tensor_scalar(out, in_, bias, None, op0=ALU.add)

    def copy_op(out, in_):
        fd = out.free_size()
        if eng_ns["act"] + _act_cost(fd) <= eng_ns["dve"] + _dve_cost(fd):
            eng_ns["act"] += _act_cost(fd)
            nc.scalar.copy(out=out, in_=in_)
        else:
            eng_ns["dve"] += _dve_cost(fd)
            nc.vector.tensor_copy(out, in_)

    # pending[t] = {(g, h): (pr_tile, sq_tile)}; selector MMs deferred one
    # dtile so the PE tail stays dense; stats drain every 2 dtiles.
    pending = []
    stat_units = {}  # current 2-dtile group's PSUM stat units

    def emit_selectors(t, prsq):
        if t % 2 == 0:
            stat_units["mean"] = mstp.tile(
                [128, TILE], f32, tag="mu", name=f"mu_{t}"
            )
            stat_units["sq"] = sstp.tile(
                [128, TILE], f32, tag="su", name=f"squ_{t}"
            )
        mu, su = stat_units["mean"], stat_units["sq"]
        for h in range(2):
            q = 2 * (t % 2) + h
            for gi in range(2):
                nc.tensor.matmul(
                    out=mu[32 * q : 32 * q + 8, :], lhsT=selm,
                    rhs=prsq[(gi, h)][0], start=(gi == 0), stop=(gi == 1),
                    tile_position=(0, 32 * q),
                )
                nc.tensor.matmul(
                    out=su[32 * q : 32 * q + 8, :], lhsT=selm,
                    rhs=prsq[(gi, h)][1], start=(gi == 0), stop=(gi == 1),
                    tile_position=(0, 32 * q),
                )

    def emit_drain(t_odd):
        """Drain stats for dtiles (t_odd-1, t_odd): fd=512 ops on 4 q-slots."""
        mu, su = stat_units["mean"], stat_units["sq"]
        mean_sb = msbp.tile([128, TILE], f32, tag="msb")
        copy_op(mean_sb, mu)
        m2 = m2p.tile([128, TILE], f32, tag="m2")
        nc.gpsimd.tensor_mul(m2, mean_sb, mean_sb)
        nvar = nvp.tile([128, TILE], f32, tag="nv")
        nc.vector.tensor_sub(nvar, m2, su)  # = mean^2 - E[p^2] (x 15/16)
        eng_ns["dve"] += _dve_cost(TILE)
        std_sb = ssbp.tile([128, TILE], f32, tag="ssb")
        nc.scalar.activation(out=std_sb, in_=nvar, func=AF.Sqrt, scale=-16.0 / 15.0)
        eng_ns["act"] += _act_cost(TILE, psum_src=False)
        for q in range(4):
            t = t_odd - 1 + q // 2
            c0 = t * DTILE + (q % 2) * TILE
            nc.sync.dma_start(
                out=meant[:, c0 : c0 + TILE], in_=mean_sb[32 * q : 32 * q + 8, :]
            )
            nc.sync.dma_start(
                out=stdt[:, c0 : c0 + TILE], in_=std_sb[32 * q : 32 * q + 8, :]
            )

    # xt loads go on the Sync DMA queue (the GpSimd queue is busy with the
    # square ops, whose semaphore waits would delay a same-queue DMA) and
    # are prefetched one dtile ahead.
    xt_tiles = {}

    def load_xt(t):
        if t >= n_dt or t in xt_tiles:
            return
        xt_t = xp.tile([128, 2, TILE], f16, tag="xt")
        nc.sync.dma_start(
            out=xt_t,
            in_=xt[:, t * DTILE : (t + 1) * DTILE].rearrange(
                "p (h n) -> p h n", n=TILE
            ),
        )
        xt_tiles[t] = xt_t

    drain_due = [None]
    load_xt(0)
    for t in range(n_dt):
        xt_t = xt_tiles.pop(t)
        h3s = {}
        for d in range(NPAIR // 2):
            js = (2 * d, 2 * d + 1)
            # L1 duo: the two pairs sit on different row groups, so emitting
            # their matmuls back-to-back lets the streams co-issue (the
            # serialized LDWEIGHTS tax stays, but the streams overlap).
            h1d = {}
            for jj in js:
                h1d[jj] = h1p.tile(
                    [128, 2, TILE], f16, tag="h1", name=f"h1_{jj}"
                )
            for h in range(2):
                us = {}
                for jj in js:
                    k = 32 * (jj % 4)
                    us[jj] = php.tile(
                        [128, TILE], f32, tag="ph", name=f"ph1_{jj}_{h}"
                    )
                    nc.tensor.matmul(
                        out=us[jj], lhsT=w1v[k : k + 32, jj, :],
                        rhs=xt_t[k : k + 32, h, :], start=True, stop=True,
                        tile_position=(k, 0),
                    )
                for jj in js:
                    epilogue(
                        h1d[jj][:, h, :], us[jj],
                        cwb[:, OFF_B1 + jj : OFF_B1 + jj + 1], relu=True,
                    )

            if d == 0:
                load_xt(t + 1)  # prefetch next dtile's input
            if d == 1 and pending:  # deferred selector MMs (dense region)
                tsel, prsq_p = pending.pop(0)
                emit_selectors(tsel, prsq_p)
                if tsel % 2 == 1:
                    drain_due[0] = tsel
            if d == 2 and drain_due[0] is not None:
                emit_drain(drain_due[0])
                drain_due[0] = None

            # L2 for both pairs, then L3 for both pairs: single pair-
            # blockdiag K=128 matmuls (fewest PE slots); the cross-pair
            # interleave keeps two matmul-slots of slack between each
            # epilogue and the matmul that consumes its output.
            h2d = {}
            for j in js:
                h2d[j] = h2p.tile([128, 2, TILE], f16, tag="h2", name=f"h2_{j}")
                for h in range(2):
                    u2 = php.tile(
                        [128, TILE], f32, tag="ph", name=f"ph2_{j}_{h}"
                    )
                    nc.tensor.matmul(
                        out=u2, lhsT=w2v[:, j, :], rhs=h1d[j][:, h, :],
                        start=True, stop=True,
                    )
                    epilogue(
                        h2d[j][:, h, :], u2,
                        cwb[:, OFF_B2 + j : OFF_B2 + j + 1], relu=True,
                    )
            for j in js:
                h3 = h3p.tile([128, 2, TILE], f16, tag="h3", name=f"h3_{j}")
                for h in range(2):
                    u3 = php.tile(
                        [128, TILE], f32, tag="ph", name=f"ph3_{j}_{h}"
                    )
                    nc.tensor.matmul(
                        out=u3, lhsT=w3v[:, j, :], rhs=h2d[j][:, h, :],
                        start=True, stop=True,
                    )
                    epilogue(
                        h3[:, h, :], u3,
                        cwb[:, OFF_B3 + j : OFF_B3 + j + 1], relu=True,
                    )
                h3s[j] = h3

        # L4 dense phase: 4 pairs col-packed per (group, half) bank
        prsq = {}
        for g in range(2):
            for h in range(2):
                p4 = p4p.tile([128, TILE], f32, tag="p4", name=f"p4_{g}_{h}")
                for q in range(4):
                    j = 4 * g + q
                    nc.tensor.matmul(
                        out=p4[32 * q : 32 * q + 32, :], lhsT=w4v[:, j, :],
                        rhs=h3s[j][:, h, :], start=True, stop=True,
                        tile_position=(0, 32 * q),
                    )
                prt = prp.tile([128, TILE], f16, tag="pr")
                epilogue(prt, p4, cwb[:, OFF_B4 + g : OFF_B4 + g + 1], relu=False)
                sqt = sqp.tile([128, TILE], f16, tag="sq")
                nc.gpsimd.tensor_mul(sqt, prt, prt)
                prsq[(g, h)] = (prt, sqt)
        pending.append((t, prsq))

    for t, prsq in pending:  # drain remaining deferred stats
        emit_selectors(t, prsq)
        if t % 2 == 1:
            drain_due[0] = t
    if drain_due[0] is not None:
        emit_drain(drain_due[0])


def build(b_core=B_CORE, num_devices=N_CORES):
    from contextlib import ExitStack

    import concourse.bacc as bacc
    import concourse.tile as tile
    from concourse import mybir

    f32 = mybir.dt.float32
    f16 = mybir.dt.float16
    nc = bacc.Bacc(
        "TRN2", target_bir_lowering=False, debug=False, num_devices=num_devices
    )
    xt = nc.dram_tensor("xt", [128, b_core], f16, kind="ExternalInput").ap()
    wr = nc.dram_tensor("wpackr", [128, WR], f16, kind="ExternalInput").ap()
    wb = nc.dram_tensor("wpackb", [128, WB], f32, kind="ExternalInput").ap()
    meant = nc.dram_tensor("meant", [8, b_core], f32, kind="ExternalOutput").ap()
    stdt = nc.dram_tensor("stdt", [8, b_core], f32, kind="ExternalOutput").ap()
    with tile.TileContext(nc) as tc:
        with ExitStack() as ctx:
            _emit(tc, ctx, xt, wr, wb, meant, stdt, b_core)
    nc.compile()
    return nc


_NC_CACHE = {}


def kernel(x, W1, b1, W2, b2, W3, b3, W4, b4):
    from concourse.bass_utils import run_bass_kernel_spmd

    key = ("full", B_CORE)
    if key not in _NC_CACHE:
        _NC_CACHE[key] = build(B_CORE, N_CORES)
    nc = _NC_CACHE[key]

    xt_per_core, wpackr, wpackb = pack_inputs(
        np.asarray(x), np.asarray(W1), np.asarray(b1), np.asarray(W2),
        np.asarray(b2), np.asarray(W3), np.asarray(b3), np.asarray(W4),
        np.asarray(b4),
    )
    in_maps = [
        {"xt": xt_per_core[c], "wpackr": wpackr, "wpackb": wpackb}
        for c in range(N_CORES)
    ]
    res = run_bass_kernel_spmd(nc, in_maps, list(range(N_CORES))).results
    mean = np.concatenate([res[c]["meant"] for c in range(N_CORES)], axis=1).T
    std = np.concatenate([res[c]["stdt"] for c in range(N_CORES)], axis=1).T
    return np.ascontiguousarray(mean), np.ascontiguousarray(std)
